# revision 15
# baseline (speedup 1.0000x reference)
"""DispersionLoss (InfoNCE_l2 variant) on 8 Trainium2 NeuronCores.

Computes  log( E_{i!=j}[ exp(-||z_i - z_j||^2 / tau) ] )  for z [8192, 512] fp32.

Strategy
--------
Let y = z * sqrt(2/tau), sqy_i = ||y_i||^2. Then
    exp(-||z_i-z_j||^2/tau) = exp(y_i.y_j) * exp(-sqy_i/2) * exp(-sqy_j/2)
(the relu clamp in the reference only matters on the diagonal, which we mask).

The 8192x8192 pair matrix is tiled into a 16x16 grid of 512x512 blocks.
Using symmetry, each unordered off-diagonal block pair is computed once:
core c owns block-rows {c, c+8} and computes blocks
    (c,   c+d) for d=0..8   and   (c+8, c+8+d mod 16) for d=0..7
which partitions { diag blocks } + { unordered pairs } exactly across 8 cores
(17 block-tiles per core). Off-diag block sums get host weight 2, diag blocks
weight 1 (their true diagonal is masked on-device via an identity-matmul that
adds -50 to the pre-exp argument).

SPMD trick: every core receives y^T with its columns *rotated* by 512*c, so
the schedule (which local column block pairs with which local lhs block) is
identical on every core; only the data differs. The lhsT tiles are slices of
the same rotated y^T already resident in SBUF (local blocks L0 and L8).

Engine split per 512x512 block-tile (a "quad" of 4 psum banks):
  - TensorE: 16 bf16 matmuls (K=128) accumulate G = y_i.y_j into a
    [128, 2048] psum tile (+1 identity-matmul per bank on diag tiles).
  - ScalarE: one Exp activation over the whole [128, 2048] psum tile
    into a bf16 SBUF tile E (pure exp).
  - VectorE: one 2x-mode multiply EW = E * A_colblock over banks 0-2
    (A_j = exp(-sqy_j/2) broadcast), then row-sum reduces of those banks
    into a [128, 51] stats buffer (host applies the a_i row factor there).
  - TensorE again: bank 3 is column-summed instead: a K=128/M=2 matmul over
    E with the Dekker-split row factor a_i as stationary weights, written
    into the (now dead) psum tile; ScalarE copies the [2, 512] result into
    an SBUF staging row (A_j factors out of column sums, so the host applies
    it there).
  - Host: applies the remaining factors / weights and log(sum/(N*(N-1))).

This splits the O(N^2) post-exp reduction work between VectorE (which has no
fast mode for TENSOR_REDUCE) and TensorE, keeping both below the main matmul
cost. The y input is laid out [16, 128, 4*512] (column-block major) so each
512KB column block is one dense DMA. Warm-up matmuls on memset data run while
the DMAs stream so the PE's HAM clock gate is open when real matmuls start.
"""

import math

import numpy as np
import ml_dtypes

TAU = 100.0
N = 8192
DIM = 512
NCORES = 8
BLK = 512          # block size (rows/cols of a block-tile)
NBLK = 16          # number of 512-blocks along each axis
P = 128
KCH = 4            # contraction chunks of 128
NQ = 17            # block-tiles per core
DIAG_QUADS = (0, 9)
DIAG_NEG = -50.0   # added to pre-exp argument on the true diagonal
N_WARMUP_MM = 26

_cache = {}


def _build_nc():
    import concourse.bacc as bacc
    import concourse.mybir as mybir
    from concourse.tile import TileContext

    bf16 = mybir.dt.bfloat16
    f32 = mybir.dt.float32
    Exp = mybir.ActivationFunctionType.Exp
    mult = mybir.AluOpType.mult
    X = mybir.AxisListType.X

    nc = bacc.Bacc(trn_type="TRN2")

    y = nc.dram_tensor("y", [NBLK, P, KCH * BLK], bf16, kind="ExternalInput")
    acol = nc.dram_tensor("acol", [4, P, 4 * BLK], bf16, kind="ExternalInput")
    apair = nc.dram_tensor("apair", [P, 16], bf16, kind="ExternalInput")
    ident = nc.dram_tensor("ident", [P, P], bf16, kind="ExternalInput")
    dpat = nc.dram_tensor("dpat", [P, 4 * BLK], bf16, kind="ExternalInput")
    stats = nc.dram_tensor("stats", [P, 3 * NQ], f32, kind="ExternalOutput")
    csums = nc.dram_tensor("csums", [2, NQ * BLK], f32, kind="ExternalOutput")

    # block-tile schedule: (lhs block index {0: local L0, 1: local L8}, local
    # col block, is_diag). Identical on every core thanks to the rotation.
    quads = (
        [(0, 0, True)]
        + [(0, L, False) for L in range(1, 9)]
        + [(1, 8, True)]
        + [(1, L, False) for L in range(9, 16)]
    )

    with TileContext(nc) as tc:
        with (
            tc.tile_pool(name="persist", bufs=1) as pp,
            tc.tile_pool(name="equad", bufs=3) as ep,
            tc.tile_pool(name="psum", bufs=2, space="PSUM") as psp,
        ):
            rhs = [
                pp.tile([P, KCH * BLK], bf16, tag=f"rhs_{L}", name=f"rhs_{L}")
                for L in range(NBLK)
            ]
            acol_t = [
                pp.tile([P, 4 * BLK], bf16, tag=f"acol_{i}", name=f"acol_{i}")
                for i in range(4)
            ]
            apair_t = pp.tile([P, 16], bf16, tag="apair", name="apair_t")
            ident_t = pp.tile([P, P], bf16, tag="ident", name="ident_t")
            dpat_t = pp.tile([P, 4 * BLK], bf16, tag="dpat", name="dpat_t")
            stats_t = pp.tile([P, 3 * NQ], f32, tag="stats", name="stats_t")
            csb_t = pp.tile([2, NQ * BLK], f32, tag="csb", name="csb_t")
            wsrc_t = pp.tile([P, BLK], bf16, tag="wsrc", name="wsrc_t")

            # PE warm-up on memset data (no DMA dependency): opens the HAM
            # clock gate while the first column blocks stream in.
            nc.gpsimd.memset(wsrc_t[:], 0.0)
            wps = psp.tile([P, 4 * BLK], f32, tag="ps", name="warm_ps")
            for i in range(N_WARMUP_MM):
                nc.tensor.matmul(
                    wps[:, :BLK], wsrc_t[:, :P], wsrc_t[:], start=True, stop=True
                )

            nc.sync.dma_start(rhs[0][:], y[0])
            nc.sync.dma_start(ident_t[:], ident[:, :])
            nc.sync.dma_start(dpat_t[:], dpat[:, :])
            nc.sync.dma_start(apair_t[:], apair[:, :])
            nc.sync.dma_start(acol_t[0][:], acol[0])
            nc.sync.dma_start(rhs[1][:], y[1])
            nc.sync.dma_start(rhs[2][:], y[2])
            nc.sync.dma_start(acol_t[1][:], acol[1])
            nc.sync.dma_start(rhs[3][:], y[3])
            nc.sync.dma_start(rhs[4][:], y[4])
            nc.sync.dma_start(acol_t[2][:], acol[2])
            nc.sync.dma_start(rhs[5][:], y[5])
            nc.sync.dma_start(acol_t[3][:], acol[3])
            for L in range(6, NBLK):
                nc.sync.dma_start(rhs[L][:], y[L])

            for q, (lhs_idx, L, is_diag) in enumerate(quads):
                lhsrc = rhs[0] if lhs_idx == 0 else rhs[8]
                ps = psp.tile([P, 4 * BLK], f32, tag="ps", name=f"ps_{q}")
                for rt_ in range(4):
                    seg = ps[:, rt_ * BLK : (rt_ + 1) * BLK]
                    for k in range(KCH):
                        nc.tensor.matmul(
                            seg,
                            lhsrc[:, k * BLK + rt_ * P : k * BLK + (rt_ + 1) * P],
                            rhs[L][:, k * BLK : (k + 1) * BLK],
                            start=(k == 0),
                            stop=(k == KCH - 1) and not is_diag,
                        )
                    if is_diag:
                        nc.tensor.matmul(
                            seg,
                            ident_t[:],
                            dpat_t[:, rt_ * BLK : (rt_ + 1) * BLK],
                            start=False,
                            stop=True,
                        )
                e = ep.tile([P, 4 * BLK], bf16, tag="e", name=f"e_{q}")
                nc.scalar.activation(e[:], ps[:], Exp)
                # weight banks 0-2 by A_j (same column block for all row
                # subtiles: broadcast the 512-wide slice across the banks)
                ew = ep.tile([P, 3 * BLK], bf16, tag="ew", name=f"ew_{q}")
                a_b = acol_t[L // 4][:, None, (L % 4) * BLK : (L % 4 + 1) * BLK]
                nc.vector.tensor_tensor(
                    ew[:].rearrange("p (r b) -> p r b", r=3),
                    e[:, : 3 * BLK].rearrange("p (r b) -> p r b", r=3),
                    a_b.to_broadcast((P, 3, BLK)),
                    mult,
                )
                # banks 0-2: VectorE row-sums (a_i applied on host)
                for rt_ in range(3):
                    nc.vector.reduce_sum(
                        stats_t[:, 3 * q + rt_ : 3 * q + rt_ + 1],
                        ew[:, rt_ * BLK : (rt_ + 1) * BLK],
                        axis=X,
                    )
                # bank 3: TensorE column-sum of E with a_i (Dekker hi/lo) as
                # the stationary operand, into the now-dead psum tile, then
                # a ScalarE copy to SBUF staging (A_j applied on host)
                rt = 4 * lhs_idx + 3
                nc.tensor.matmul(
                    ps[0:2, :BLK],
                    apair_t[:, 2 * rt : 2 * rt + 2],
                    e[:, 3 * BLK : 4 * BLK],
                    start=True,
                    stop=True,
                )
                nc.scalar.copy(
                    csb_t[:, q * BLK : (q + 1) * BLK], ps[0:2, :BLK]
                )

            nc.sync.dma_start(stats[:, :], stats_t[:])
            nc.sync.dma_start(csums[:, :], csb_t[:])

    nc.compile()
    return nc


def _host_inputs(z: np.ndarray):
    """Build the per-core input maps from the full z [8192, 512] fp32."""
    bf16 = ml_dtypes.bfloat16
    z64 = z.astype(np.float64)
    s = math.sqrt(2.0 / TAU)
    yT64 = (z64 * s).T  # [512, 8192]
    sqy64 = (2.0 / TAU) * np.sum(z64 * z64, axis=1)  # [8192]
    v64 = -0.5 * sqy64  # -sqy_j / 2

    ident = np.eye(P, dtype=np.float32).astype(bf16)
    dpat = np.zeros((P, 4 * BLK), dtype=np.float32)
    for rt_ in range(4):
        for p in range(P):
            dpat[p, rt_ * BLK + rt_ * P + p] = DIAG_NEG
    dpat = dpat.astype(bf16)

    in_maps = []
    amaps = []
    acol64s = []
    for c in range(NCORES):
        yr = np.roll(yT64, -BLK * c, axis=1).astype(np.float32).astype(bf16)
        # [512, 8192] -> [L=16, p=128, k=4, c=512] -> [16, 128, 2048]
        yl = np.ascontiguousarray(
            yr.reshape(KCH, P, NBLK, BLK).transpose(2, 1, 0, 3).reshape(
                NBLK, P, KCH * BLK
            )
        )

        vr = np.roll(v64, -BLK * c)
        acol = np.ascontiguousarray(
            np.broadcast_to(
                np.exp(vr).astype(np.float32).astype(bf16)[None, :], (P, N)
            ).reshape(P, 4, 4 * BLK).transpose(1, 0, 2)
        )

        # a_i = exp(-sqy_i/2) per local row, Dekker split hi/lo in bf16
        a_rows64 = np.empty((8, P), dtype=np.float64)
        apair = np.zeros((P, 16), dtype=np.float32)
        for rt in range(8):
            base = BLK * (c + 8 * (rt // 4)) + (rt % 4) * P
            a = np.exp(v64[base : base + P])
            a_rows64[rt] = a
            hi = a.astype(np.float32).astype(bf16)
            lo = (a - hi.astype(np.float64)).astype(np.float32).astype(bf16)
            apair[:, 2 * rt] = hi.astype(np.float32)
            apair[:, 2 * rt + 1] = lo.astype(np.float32)
        apair = apair.astype(bf16)

        # host-side row factor for the VectorE-reduced banks (0-2 per quad)
        amap = np.empty((P, 3 * NQ), dtype=np.float64)
        for q in range(NQ):
            lhs_idx = 0 if q < 9 else 1
            for rt_ in range(3):
                amap[:, 3 * q + rt_] = a_rows64[4 * lhs_idx + rt_]
        amaps.append(amap)
        acol64s.append(np.exp(vr))

        in_maps.append(
            {
                "y": yl,
                "acol": acol,
                "apair": apair,
                "ident": ident,
                "dpat": dpat,
            }
        )
    return in_maps, amaps, acol64s


QUAD_L = [0, 1, 2, 3, 4, 5, 6, 7, 8, 8, 9, 10, 11, 12, 13, 14, 15]


def _reduce(results, amaps, acol64s) -> np.ndarray:
    wq = np.array([1.0 if q in DIAG_QUADS else 2.0 for q in range(NQ)])
    total = 0.0
    for out_map, amap, a64 in zip(results, amaps, acol64s):
        st = out_map["stats"].astype(np.float64)  # [P, 3*NQ]
        per_t = (st * amap).sum(axis=0).reshape(NQ, 3).sum(axis=1)  # [NQ]
        cs = out_map["csums"].astype(np.float64)  # [2, NQ*BLK]
        csq = cs.sum(axis=0).reshape(NQ, BLK)  # hi+lo rows -> [NQ, BLK]
        aw = np.stack([a64[L * BLK : (L + 1) * BLK] for L in QUAD_L])
        per_c = (csq * aw).sum(axis=1)  # [NQ]
        total += (wq * (per_t + per_c)).sum()
    mean = total / (float(N) * float(N - 1))
    return np.array(math.log(mean), dtype=np.float32)


def run(z: np.ndarray, trace: bool = False, tmpdir=None):
    from concourse.bass_utils import run_bass_kernel_spmd

    if "nc" not in _cache:
        _cache["nc"] = _build_nc()
    nc = _cache["nc"]
    in_maps, amaps, acol64s = _host_inputs(np.asarray(z, dtype=np.float32))
    res = run_bass_kernel_spmd(
        nc, in_maps, core_ids=list(range(NCORES)), trace=trace, tmpdir=tmpdir
    )
    return _reduce(res.results, amaps, acol64s), res


def kernel(z: np.ndarray) -> np.ndarray:
    out, _ = run(z, trace=False)
    return out
